# revision 56
# baseline (speedup 1.0000x reference)
"""Trainium2 Bass kernel for nn_BitLinear (LayerNorm -> 1.58-bit BitLinear).

Math notes
----------
Reference computes, per the module:
    xn    = LN(x) * ln_gamma + ln_beta            (eps = 1e-3)
    beta  = mean(|W|);  w_q = clip(round(W / (beta + 1e-5)), -1, 1)
    gamma = max(|xn|)   (global absmax)
    xq    = clip(xn * 128 / gamma, -128 + 1e-5, 128 - 1e-5)
    y     = (xq @ w_q) * (gamma * beta / 128)

The gamma factor cancels exactly: (xn*128/gamma) @ w_q * (gamma*beta/128)
== (xn @ w_q) * beta.  The clip only affects elements within relative
7.8e-8 of the global absmax -- far below f32 matmul roundoff.  So the
kernel computes y = (LN(x) @ w_q) * beta, fully data-parallel over
tokens (no collectives).

w_q is ternary: w_q = sign(W) * 1[|W| > c] with c = 0.5*(beta+1e-5).
The kernel stores wq' = 0.5*w_q via one fused DVE op per k-block:
    wq = (|W| is_gt c) * sgnh,   sgnh = (W>=0)-0.5 in {-.5,+.5}
(the 2x is folded into the output scale 2*beta).  All compares are f32:
a bf16 compare would misclassify ~300 weights near the threshold.

LN normalization scale folds into the epilogue: xn = (x - mu) in bf16,
and esc[t] = rsqrt(var+eps)[t] * 2*beta scales each output row.

Sharding: data-parallel over the 32768 tokens, 4096 per core; weight
replicated.  y is written bf16 (halves drain traffic; ~1e-3 extra
rel-err) and upcast to f32 on the host.

Schedule (v2, rebuilt from trace analysis of the previous kernel):
  * Measured engine rates: MM N=512 gap 216ns warm / 259ns in the P0
    power state (run-to-run chip power state; uncontrollable); PE
    transposes 56ns; DVE f32 pass over [128,1024] ~0.7us; ACT pass
    ~1.1us; HWDGE rings 125-245 B/ns each (HBM-stack contention with
    the other 7 cores' prologues); NEFF preamble ~8.5us.  Per-engine
    instruction order is STATIC (fixed by Tile's cost-model
    simulation), so emission order must be correct for slow AND fast
    DMA weather -- the runtime does not re-dispatch by readiness.
  * Prologue DMA: q1 (sync): x0, k0, k1, k2, x1, x2...
                  q10 (scalar): k3..k7, then y drains.
    (gpsimd SWDGE measured ~2x slower -- not used.)
  * W prep: super-0 stats/xn/transposes/copies first in the static
    order (x0 is q1's first transfer -- always ready first), then
    per-chunk sgnh (DVE) + |W|+row-sum (3 early chunks fused on DVE
    in-place, 5 on ACT, the last-landing two to scratch), in
    ring-interleaved landing order so a slow ring costs at most one
    cross-ring head-of-line wait.  After each chunk's sgnh the PE
    transposes it into ps_dummy: warmup that stretches with the
    actual DMA rate, keeping the HAM clock warm however slow the
    prologue runs.  beta -> c closes ~2us after the last W byte.
  * Ternarize trickles per k (fused DVE stt ~1.3us/k) interleaved
    with the first super's matmuls k-by-k (first MM ~33us vs 42 in
    the parent kernel; x1's DMA issue is emitted after the trickle so
    the scheduler cannot hoist super-1 stats ahead of the beta chain).
  * Steady loop per tile: M(j)i then T(j+1)i; xT PSUM->SBUF copies on
    DVE during back(j); ACT does xn + epilogue; per-super y drains
    (bf16, 0.5 MiB) on q10, x loads (1 MiB) on q1.  x DMA issues run
    3 supers ahead but front_stats only 2, so bn_stats never block
    the next xT copies in the DVE FIFO waiting for an unlanded x.
  * Final super runs h-major with a separate 1-bank PSUM tile per
    half (a shared tile WAR-serializes h1 matmuls behind h0's
    epilogue read), per-half drains on both rings.
  * The beta chain reads the ones-matmul total straight from PSUM
    (tensor_scalar accepts PSUM in0), skipping a serialized DVE copy
    + sem hop on the path gating the first matmul.
  * Measured: 166.6-166.8us warm clock / ~196us in the P0 power
    state (parent kernel: 171-175 / 199-202).  PE stalls 4.9us; the
    floor is ~8.5 preamble + ~24 ramp (5 MB prologue DMA + global
    beta dependency) + 124.9 PE work + ~4 tail = ~161us.
"""

import numpy as np

B, S, D, U = 4, 8192, 1024, 1024
N_CORES = 8
TOK = (B * S) // N_CORES  # 4096 tokens per core
P = 128
KB = D // P               # 8 contraction blocks
NTILES = TOK // P         # 32 token tiles per core
SUPER = 2                 # token tiles per DMA transfer (1 MiB chunks)
NJ = NTILES // SUPER      # 16 super-tiles
N_DUMMY1 = 95             # PE warmup transposes until T0's xn is ready
N_DUMMY2 = 45             # ... after the W block, before the ones-matmul
N_DUMMY3 = 20             # ... between the ones-matmul and first MM
LN_EPS = 1e-3
EPS = 1e-5

# W prologue rides both HWDGE rings (SWDGE measured ~2x slower):
#   q1 (sync):    x0, k0, k1, k2, then x1, x2, ...
#   q10 (scalar): k3, k4, k5, k6, k7, then y drains
W_SYNC = (0, 1, 2, 7)     # q1, behind x0's first half
W_SCALAR = (3, 4, 5, 6)   # q10, behind x0's second half
W_LAND = (3, 0, 4, 1, 5, 2, 6, 7)   # ring-interleaved landing order
DVE_ABS = (3, 0, 4)       # |W| via fused DVE stt+accum (offloads ACT)
SCRATCH_ABS = (6, 7)      # last-landing ACT chunks: abs to scratch

_NC_CACHE = {}


def _build(apply_gamma: bool, apply_beta: bool):
    """Build the single-core Bass program (SPMD: same NEFF on all 8 cores)."""
    import concourse.bacc as bacc
    import concourse.mybir as mybir
    import concourse.tile as tile
    from concourse.bass import ts
    from concourse.masks import make_identity

    fp32 = mybir.dt.float32
    bf16 = mybir.dt.bfloat16
    AF = mybir.ActivationFunctionType
    OP = mybir.AluOpType

    nc = bacc.Bacc()
    x_h = nc.dram_tensor("x", [TOK, D], fp32, kind="ExternalInput")
    w_h = nc.dram_tensor("weight", [D, U], fp32, kind="ExternalInput")
    g_h = (
        nc.dram_tensor("ln_gamma", [D], fp32, kind="ExternalInput")
        if apply_gamma
        else None
    )
    lb_h = (
        nc.dram_tensor("ln_beta", [D], fp32, kind="ExternalInput")
        if apply_beta
        else None
    )
    y_h = nc.dram_tensor("y", [TOK, U], bf16, kind="ExternalOutput")

    with tile.TileContext(nc) as tc:
        with (
            tc.tile_pool(name="singles", bufs=1) as singles,
            tc.tile_pool(name="xin", bufs=5) as xin_pool,
            tc.tile_pool(name="xn", bufs=7) as xn_pool,
            tc.tile_pool(name="xt", bufs=7) as xt_pool,
            tc.tile_pool(name="yout", bufs=3) as y_pool,
            tc.tile_pool(name="stats", bufs=8) as stats_pool,
            tc.tile_pool(name="ps_t", bufs=3, space="PSUM") as ps_t_pool,
            tc.tile_pool(name="ps_y", bufs=2, space="PSUM") as ps_y_pool,
        ):
            # ---- constants ----
            ident = singles.tile([P, P], bf16)
            make_identity(nc, ident)
            eps_t = singles.tile([P, 1], fp32)
            nc.vector.memset(eps_t, LN_EPS)
            ones_f32 = singles.tile([P, P], fp32)
            nc.vector.memset(ones_f32, 1.0)

            # ---- DMA issue order defines ring FIFO order ----
            w_view = w_h[:, :].rearrange("(ko ki) u -> ki ko u", ki=P)
            x_view = x_h[:, :].rearrange("(o p) d -> p o d", p=P)
            y_view = y_h[:, :].rearrange("(o p) u -> p o u", p=P)

            def issue_x(j, eng=None):
                x_sb = xin_pool.tile([P, SUPER, D], fp32, name="x_sb")
                (eng or nc.sync).dma_start(
                    out=x_sb, in_=x_view[:, j * SUPER : (j + 1) * SUPER, :]
                )
                return x_sb

            # Super-0 rides BOTH ring heads as two half-tile transfers:
            # both tiles land ~2.3us after the rings start, so all of
            # super-0's front work (stats/xn/transposes/copies) clears
            # DVE/ACT before the W tail arrives, leaving the engines
            # free for the beta chain.  Ring bytes stay balanced
            # (2.5 MB each: half-x0 + 2 MB of W per ring).
            w_sb = singles.tile([P, KB, U], fp32)
            x_supers = {}
            x0a = xin_pool.tile([P, 1, D], fp32, name="x0a")
            nc.sync.dma_start(out=x0a, in_=x_view[:, 0:1, :])
            x0b = xin_pool.tile([P, 1, D], fp32, name="x0b")
            nc.scalar.dma_start(out=x0b, in_=x_view[:, 1:2, :])
            for k in W_SCALAR:
                nc.scalar.dma_start(out=w_sb[:, k, :], in_=w_view[:, k, :])
            for k in W_SYNC:
                nc.sync.dma_start(out=w_sb[:, k, :], in_=w_view[:, k, :])
            # (x1's issue is emitted after the ternarize trickle: the ring
            # FIFO order is identical, but the scheduler's cost model then
            # sees x1 landing late and won't hoist super-1 stats ahead of
            # the beta chain in the static DVE program.)

            if apply_gamma:
                g_sb = singles.tile([P, KB], fp32)
                nc.scalar.dma_start(
                    out=g_sb, in_=g_h[:].rearrange("(ko ki) -> ki ko", ki=P)
                )
            if apply_beta:
                lb_f32 = singles.tile([P, KB], fp32)
                nc.scalar.dma_start(
                    out=lb_f32, in_=lb_h[:].rearrange("(ko ki) -> ki ko", ki=P)
                )
                lb_sb = singles.tile([P, KB], bf16)
                nc.vector.tensor_copy(out=lb_sb, in_=lb_f32)

            # ---- W prep: sgnh on DVE, |W|+row-sum accum on ACT ----
            sgnh = singles.tile([P, KB, U], bf16)
            asum = singles.tile([P, KB], fp32)
            scratch = singles.tile([P, 2, U], fp32)
            abs_src = {}  # k -> AP holding |W| for the ternarize compare

            def emit_sgnh(k):
                # (W>=0)-0.5 in {-.5,+.5}, exact in bf16  (DVE)
                nc.vector.tensor_scalar(
                    out=sgnh[:, k, :], in0=w_sb[:, k, :], scalar1=0.0,
                    scalar2=0.5, op0=OP.is_ge, op1=OP.subtract,
                )
                if apply_gamma and not apply_beta:
                    nc.vector.tensor_scalar(
                        out=sgnh[:, k, :], in0=sgnh[:, k, :],
                        scalar1=g_sb[:, k : k + 1], scalar2=None, op0=OP.mult,
                    )

            def emit_abs(k):
                # |W| with row-sum accum.  DVE chunks: in-place abs_max
                # (0.68us vs ACT's 1.4); ACT chunks: Abs, the last-landing
                # ones to scratch so ACT needn't wait on DVE's sgnh read.
                if k in DVE_ABS and not (apply_gamma and not apply_beta):
                    # |W| = (2W) * sgnh exactly, row-sum accumulated: one
                    # fused DVE pass, in place (sgnh emitted just before;
                    # skipped when sgnh carries the ln_gamma fold)
                    nc.vector.scalar_tensor_tensor(
                        out=w_sb[:, k, :], in0=w_sb[:, k, :], scalar=2.0,
                        in1=sgnh[:, k, :], op0=OP.mult, op1=OP.mult,
                        accum_out=asum[:, k : k + 1],
                    )
                    abs_src[k] = w_sb[:, k, :]
                    return
                if k in SCRATCH_ABS:
                    i = SCRATCH_ABS.index(k)
                    dst = scratch[:, i, :]
                else:
                    dst = w_sb[:, k, :]
                nc.scalar.activation(
                    out=dst, in_=w_sb[:, k, :], func=AF.Abs,
                    accum_out=asum[:, k : k + 1],
                )
                abs_src[k] = dst

            # ---- LN stats on DVE; the normalize pass rides ACT ----
            def front_stats(x_sb, i):
                xt_ = x_sb[:, i, :]
                st = stats_pool.tile([P, 2, 6], fp32, tag="st")
                xr = xt_.rearrange("p (n f) -> p n f", f=512)
                nc.vector.bn_stats(out=st[:, 0, :], in_=xr[:, 0, :])
                nc.vector.bn_stats(out=st[:, 1, :], in_=xr[:, 1, :])
                mv = stats_pool.tile([P, 2], fp32, tag="mv")
                nc.vector.bn_aggr(out=mv, in_=st)
                nb = stats_pool.tile([P, 1], fp32, tag="nb")
                nc.vector.tensor_scalar(
                    out=nb, in0=mv[:, 0:1], scalar1=-1.0, scalar2=None,
                    op0=OP.mult,
                )
                # xn = x - mu (bf16); rsqrt scale folds into the epilogue
                xn = xn_pool.tile([P, D], bf16)
                nc.scalar.activation(
                    out=xn, in_=xt_, func=AF.Identity, bias=nb, scale=1.0
                )
                # sq = sqrt(var + eps) (tiny, ACT)
                sq = stats_pool.tile([P, 1], fp32, tag="sq")
                nc.scalar.activation(
                    out=sq, in_=mv[:, 1:2], func=AF.Sqrt, bias=eps_t, scale=1.0
                )
                return xn, sq

            # Pre-warm the SQRT ACT table (1.3us load) before it's needed
            # on the stats critical path.
            sq_warm = singles.tile([P, 1], fp32)
            nc.scalar.activation(
                out=sq_warm, in_=eps_t, func=AF.Sqrt, bias=eps_t, scale=1.0
            )

            # ---- PE warmup dummies until the first W chunk lands ----
            ps_dummy = ps_t_pool.tile([P, KB, P], bf16, tag="ps_t", name="ps_d")
            for i in range(N_DUMMY1):
                nc.tensor.transpose(ps_dummy[:, i % KB, :], ident, ident)

            def warm(k):
                # PE transposes of the just-produced sgnh chunk: warmup
                # that stretches with the actual DMA rate, so HAM stays
                # at full clock however slow the prologue runs.
                for j in range(KB):
                    nc.tensor.transpose(
                        ps_dummy[:, j, :], sgnh[:, k, ts(j, P)], ident
                    )

            # ---- transposes + copies ----
            def transpose_tile(fr):
                xn, sq = fr
                ps_xt = ps_t_pool.tile([P, KB, P], bf16, tag="ps_t")
                for k in range(KB):
                    nc.tensor.transpose(ps_xt[:, k, :], xn[:, ts(k, P)], ident)
                xT = xt_pool.tile([P, KB, P], bf16)
                nc.vector.tensor_copy(out=xT, in_=ps_xt)
                return (xT, sq)

            # Super-0 stats/transposes/copies first in the static order
            # (both x0 halves are ring-head transfers, so they are ready
            # before any W chunk in every DMA weather), then W prep in
            # ring-interleaved landing order.  (Tested alternative: early
            # W chunks before stats -- measured 5us WORSE.)
            frs0 = [front_stats(x0a, 0), front_stats(x0b, 0)]
            f0 = [transpose_tile(fr) for fr in frs0]
            for k in W_LAND:
                emit_sgnh(k)
                emit_abs(k)
                warm(k)

            asum1 = singles.tile([P, 1], fp32)
            nc.vector.tensor_reduce(
                out=asum1, in_=asum, axis=mybir.AxisListType.X, op=OP.add
            )

            for i in range(N_DUMMY2):
                nc.tensor.transpose(ps_dummy[:, i % KB, :], ident, ident)

            # cross-partition total broadcast to all partitions in ONE matmul
            ps_tot = ps_y_pool.tile([P, U], fp32, tag="ps_y", name="ps_tot")
            nc.tensor.matmul(
                ps_tot[:, 0:1], lhsT=ones_f32, rhs=asum1, start=True, stop=True
            )
            # c = (beta+EPS)/2 ;  output scale 2*beta (wq holds 0.5*w_q)
            # -- both read ps_tot straight from PSUM: one fewer serialized
            # DVE op + semaphore hop on the chain gating the first matmul
            c128 = singles.tile([P, 1], fp32)
            nc.vector.tensor_scalar(
                out=c128, in0=ps_tot[:, 0:1], scalar1=0.5 / (D * U),
                scalar2=0.5 * EPS, op0=OP.mult, op1=OP.add,
            )
            bh128 = singles.tile([P, 1], fp32)
            nc.vector.tensor_scalar(
                out=bh128, in0=ps_tot[:, 0:1], scalar1=2.0 / (D * U),
                scalar2=None, op0=OP.mult,
            )

            for i in range(N_DUMMY3):
                nc.tensor.transpose(ps_dummy[:, i % KB, :], ident, ident)

            # ---- ternarize: fused (|W| is_gt c) * sgnh per k-block ----
            wq = singles.tile([P, KB, U], bf16)  # holds 0.5*w_q (*gamma)

            def emit_tern_h(k, h):
                sl = ts(h, 512)
                nc.vector.scalar_tensor_tensor(
                    out=wq[:, k, sl], in0=abs_src[k][:, sl], scalar=c128,
                    in1=sgnh[:, k, sl], op0=OP.is_gt, op1=OP.mult,
                )

            def emit_tern(k):
                nc.vector.scalar_tensor_tensor(
                    out=wq[:, k, :], in0=abs_src[k], scalar=c128,
                    in1=sgnh[:, k, :], op0=OP.is_gt, op1=OP.mult,
                )

            # ---- back side ----
            def esc_for(sq):
                # esc = rsqrt(var+eps) * 2*beta, per token (tiny DVE chain)
                esc = stats_pool.tile([P, 1], fp32, tag="esc")
                nc.vector.reciprocal(esc, sq)
                nc.vector.tensor_scalar(
                    out=esc, in0=esc, scalar1=bh128, scalar2=None, op0=OP.mult
                )
                return esc

            beff128 = None

            def epilogue(y_sb, i, ps_y, esc, h=None):
                sl = slice(None) if h is None else ts(h, 512)
                nc.scalar.mul(out=y_sb[:, i, sl], in_=ps_y[:, sl], mul=esc)
                if apply_beta:
                    nc.vector.tensor_tensor(
                        y_sb[:, i, sl], y_sb[:, i, sl], beff128[:, sl], OP.add
                    )

            def back_tile(xt_sq, y_sb, i, j):
                xT, sq = xt_sq
                last = j == NJ - 1
                esc = esc_for(sq)
                if last:
                    # h-major: each half accumulates into its OWN 1-bank
                    # PSUM tile (a shared tile would WAR-serialize h1's
                    # matmuls behind h0's epilogue read), so each half's
                    # epilogue + drain starts as soon as it closes.  The
                    # half-tiles come from the 3-deep ps_t pool (same
                    # 1-bank size as the transpose tiles, all consumed by
                    # now) so no WAR on a recent epilogue stalls the MMs.
                    for h in range(2):
                        ps_h = ps_t_pool.tile([P, 512], fp32, tag="ps_t")
                        for k in range(KB):
                            nc.tensor.matmul(
                                ps_h,
                                lhsT=xT[:, k, :],
                                rhs=wq[:, k, ts(h, 512)],
                                start=(k == 0),
                                stop=(k == KB - 1),
                            )
                        if i == SUPER - 1 and h == 1:
                            # very last half: per-quarter epilogue+drain so
                            # the final transfer is only 128 KB and its
                            # predecessor drains during the last epilogue
                            for q in range(2):
                                sl = slice(512 + q * 256, 768 + q * 256)
                                if q == 0:
                                    nc.scalar.mul(
                                        out=y_sb[:, i, sl],
                                        in_=ps_h[:, ts(q, 256)], mul=esc,
                                    )
                                else:
                                    # second quarter's epilogue on DVE so
                                    # both quarters close in parallel
                                    nc.vector.tensor_scalar(
                                        out=y_sb[:, i, sl],
                                        in0=ps_h[:, ts(q, 256)], scalar1=esc,
                                        scalar2=None, op0=OP.mult,
                                    )
                                if apply_beta:
                                    nc.vector.tensor_tensor(
                                        y_sb[:, i, sl], y_sb[:, i, sl],
                                        beff128[:, sl], OP.add,
                                    )
                                eng = nc.scalar if q == 0 else nc.sync
                                eng.dma_start(
                                    out=y_view[:, j * SUPER + i, sl],
                                    in_=y_sb[:, i, sl],
                                )
                            continue
                        sl = ts(h, 512)
                        nc.scalar.mul(out=y_sb[:, i, sl], in_=ps_h, mul=esc)
                        if apply_beta:
                            nc.vector.tensor_tensor(
                                y_sb[:, i, sl], y_sb[:, i, sl],
                                beff128[:, sl], OP.add,
                            )
                        eng = nc.scalar if h == 0 else nc.sync
                        eng.dma_start(
                            out=y_view[:, j * SUPER + i, sl],
                            in_=y_sb[:, i, sl],
                        )
                    return
                ps_y = ps_y_pool.tile([P, U], fp32, tag="ps_y")
                for k in range(KB):
                    for h in range(2):
                        nc.tensor.matmul(
                            ps_y[:, ts(h, 512)],
                            lhsT=xT[:, k, :],
                            rhs=wq[:, k, ts(h, 512)],
                            start=(k == 0),
                            stop=(k == KB - 1),
                        )
                epilogue(y_sb, i, ps_y, esc)

            def drain_y(j, y_sb):
                if j != NJ - 1:
                    nc.scalar.dma_start(
                        out=y_view[:, j * SUPER : (j + 1) * SUPER, :], in_=y_sb
                    )

            if apply_beta:
                # beff = (ln_beta @ w_q) * 2beta, broadcast to 128 partitions
                for k in range(KB):
                    emit_tern(k)
                ps_beff = ps_y_pool.tile([P, U], fp32, tag="ps_y", name="ps_bf")
                for k in range(KB):
                    for h in range(2):
                        nc.tensor.matmul(
                            ps_beff[0:1, ts(h, 512)],
                            lhsT=lb_sb[:, k : k + 1],
                            rhs=wq[:, k, ts(h, 512)],
                            start=(k == 0),
                            stop=(k == KB - 1),
                        )
                beff = singles.tile([1, U], fp32)
                nc.vector.tensor_scalar(
                    out=beff, in0=ps_beff[0:1, :], scalar1=bh128[0:1, 0:1],
                    scalar2=None, op0=OP.mult,
                )
                ps_b2 = ps_y_pool.tile([P, U], fp32, tag="ps_y")
                ones_row = singles.tile([1, P], fp32)
                nc.vector.memset(ones_row, 1.0)
                for h in range(2):
                    nc.tensor.matmul(
                        ps_b2[:, ts(h, 512)], lhsT=ones_row,
                        rhs=beff[:, ts(h, 512)], start=True, stop=True,
                    )
                beff128 = singles.tile([P, U], bf16)
                nc.vector.tensor_copy(out=beff128, in_=ps_b2)
                if apply_gamma:
                    for k in range(KB):
                        nc.vector.tensor_scalar(
                            out=wq[:, k, :], in0=wq[:, k, :],
                            scalar1=g_sb[:, k : k + 1], scalar2=None,
                            op0=OP.mult,
                        )

            # ---- first super: ternarize trickles per k, matmuls follow
            # k-by-k; super-1 stats/transposes slot into the gaps.
            x_supers[1] = issue_x(1)
            y_sb0 = y_pool.tile([P, SUPER, U], bf16)
            (xtA, sqA), (xtB, sqB) = f0
            psA = ps_y_pool.tile([P, U], fp32, tag="ps_y")
            psB = ps_y_pool.tile([P, U], fp32, tag="ps_y")
            fr1 = [None, None]
            f1 = [None, None]
            for k in range(KB):
                # ternarize per u-half so the first consumable wq slice
                # lands in ~0.7us instead of ~1.3
                for h in range(2):
                    if not apply_beta:
                        emit_tern_h(k, h)
                    for ps, xt in ((psA, xtA), (psB, xtB)):
                        nc.tensor.matmul(
                            ps[:, ts(h, 512)],
                            lhsT=xt[:, k, :],
                            rhs=wq[:, k, ts(h, 512)],
                            start=(k == 0),
                            stop=(k == KB - 1),
                        )
                # one super-1 stats block rides the trickle; the rest of
                # super-1's front work follows the last ternarize so the
                # DVE-serial tern stream stays dense.
                if k == 5:
                    fr1[0] = front_stats(x_supers[1], 0)
            escA, escB = esc_for(sqA), esc_for(sqB)
            epilogue(y_sb0, 0, psA, escA)
            f1[0] = transpose_tile(fr1[0])
            fr1[1] = front_stats(x_supers[1], 1)
            epilogue(y_sb0, 1, psB, escB)
            f1[1] = transpose_tile(fr1[1])
            drain_y(0, y_sb0)
            fronts = {1: f1}

            x_supers[2] = issue_x(2)
            x_supers[3] = issue_x(3)
            frs = {2: [front_stats(x_supers[2], i) for i in range(SUPER)]}

            # ---- steady loop, per-tile interleave: M(j)i then T(j+1)i.
            # x DMA is issued 3 supers ahead but front_stats only 2 ahead,
            # so the bn_stats never wait on an unlanded x super while the
            # next xT copies sit behind them in the DVE FIFO.
            for j in range(1, NJ):
                y_sb = y_pool.tile([P, SUPER, U], bf16)
                xts = fronts.pop(j)
                nxt = [] if j + 1 < NJ else None
                for i in range(SUPER):
                    back_tile(xts[i], y_sb, i, j)
                    if nxt is not None:
                        nxt.append(transpose_tile(frs[j + 1][i]))
                if nxt is not None:
                    del frs[j + 1]
                    fronts[j + 1] = nxt
                drain_y(j, y_sb)
                if j + 3 < NJ:
                    x_supers[j + 3] = issue_x(j + 3)
                if j + 2 < NJ:
                    frs[j + 2] = [
                        front_stats(x_supers[j + 2], i) for i in range(SUPER)
                    ]

    nc.compile()
    return nc


def _get_nc(apply_gamma: bool, apply_beta: bool):
    key = (apply_gamma, apply_beta)
    if key not in _NC_CACHE:
        _NC_CACHE[key] = _build(apply_gamma, apply_beta)
    return _NC_CACHE[key]


def _make_in_maps(x, w, g, lb, apply_gamma, apply_beta):
    xf = np.ascontiguousarray(x.reshape(B * S, D))
    in_maps = []
    for c in range(N_CORES):
        m = {
            "x": np.ascontiguousarray(xf[c * TOK : (c + 1) * TOK]),
            "weight": w,
        }
        if apply_gamma:
            m["ln_gamma"] = g
        if apply_beta:
            m["ln_beta"] = lb
        in_maps.append(m)
    return in_maps


def run(inputs, trace=False, tmpdir=None):
    """Shard, run on 8 cores, gather. Returns (y, BassKernelResults)."""
    from concourse.bass_utils import run_bass_kernel_spmd

    x = np.asarray(inputs["x"], dtype=np.float32)
    w = np.ascontiguousarray(np.asarray(inputs["weight"], dtype=np.float32))
    g = np.ascontiguousarray(np.asarray(inputs["ln_gamma"], dtype=np.float32))
    lb = np.ascontiguousarray(np.asarray(inputs["ln_beta"], dtype=np.float32))
    apply_gamma = not bool(np.all(g == 1.0))
    apply_beta = not bool(np.all(lb == 0.0))

    nc = _get_nc(apply_gamma, apply_beta)
    in_maps = _make_in_maps(x, w, g, lb, apply_gamma, apply_beta)
    res = run_bass_kernel_spmd(
        nc, in_maps, core_ids=list(range(N_CORES)), trace=trace, tmpdir=tmpdir
    )
    y = np.concatenate(
        [np.asarray(r["y"]).astype(np.float32) for r in res.results], axis=0
    )
    return y.reshape(B, S, U), res


def kernel(**inputs) -> np.ndarray:
    y, _ = run(inputs, trace=False)
    return y


# revision 57
# speedup vs baseline: 1.0091x; 1.0091x over previous
"""Trainium2 Bass kernel for nn_BitLinear (LayerNorm -> 1.58-bit BitLinear).

Math notes
----------
Reference computes, per the module:
    xn    = LN(x) * ln_gamma + ln_beta            (eps = 1e-3)
    beta  = mean(|W|);  w_q = clip(round(W / (beta + 1e-5)), -1, 1)
    gamma = max(|xn|)   (global absmax)
    xq    = clip(xn * 128 / gamma, -128 + 1e-5, 128 - 1e-5)
    y     = (xq @ w_q) * (gamma * beta / 128)

The gamma factor cancels exactly: (xn*128/gamma) @ w_q * (gamma*beta/128)
== (xn @ w_q) * beta.  The clip only affects elements within relative
7.8e-8 of the global absmax -- far below f32 matmul roundoff.  So the
kernel computes y = (LN(x) @ w_q) * beta, fully data-parallel over
tokens (no collectives).

w_q is ternary: w_q = sign(W) * 1[|W| > c] with c = 0.5*(beta+1e-5).
The kernel stores wq' = 0.5*w_q via one fused DVE op per k-block:
    wq = (|W| is_gt c) * sgnh,   sgnh = (W>=0)-0.5 in {-.5,+.5}
(the 2x is folded into the output scale 2*beta).  All compares are f32:
a bf16 compare would misclassify ~300 weights near the threshold.

LN normalization scale folds into the epilogue: xn = (x - mu) in bf16,
and esc[t] = rsqrt(var+eps)[t] * 2*beta scales each output row.

Sharding: data-parallel over the 32768 tokens, 4096 per core; weight
replicated.  y is written bf16 (halves drain traffic; ~1e-3 extra
rel-err) and upcast to f32 on the host.

Schedule (v2, rebuilt from trace analysis of the previous kernel):
  * Measured engine rates: MM N=512 gap 216ns warm / 259ns in the P0
    power state (run-to-run chip power state; uncontrollable); PE
    transposes 56ns; DVE f32 pass over [128,1024] ~0.7us; ACT pass
    ~1.1us; HWDGE rings 125-245 B/ns each (HBM-stack contention with
    the other 7 cores' prologues); NEFF preamble ~8.5us.  Per-engine
    instruction order is STATIC (fixed by Tile's cost-model
    simulation), so emission order must be correct for slow AND fast
    DMA weather -- the runtime does not re-dispatch by readiness.
  * Prologue DMA: q1 (sync): x0, k0, k1, k2, x1, x2...
                  q10 (scalar): k3..k7, then y drains.
    (gpsimd SWDGE measured ~2x slower -- not used.)
  * W prep: super-0 stats/xn/transposes/copies first in the static
    order (x0 is q1's first transfer -- always ready first), then
    per-chunk sgnh (DVE) + |W|+row-sum (3 early chunks fused on DVE
    in-place, 5 on ACT, the last-landing two to scratch), in
    ring-interleaved landing order so a slow ring costs at most one
    cross-ring head-of-line wait.  After each chunk's sgnh the PE
    transposes it into ps_dummy: warmup that stretches with the
    actual DMA rate, keeping the HAM clock warm however slow the
    prologue runs.  beta -> c closes ~2us after the last W byte.
  * Ternarize trickles per k (fused DVE stt ~1.3us/k) interleaved
    with the first super's matmuls k-by-k (first MM ~33us vs 42 in
    the parent kernel; x1's DMA issue is emitted after the trickle so
    the scheduler cannot hoist super-1 stats ahead of the beta chain).
  * Steady loop per tile: M(j)i then T(j+1)i; xT PSUM->SBUF copies on
    DVE during back(j); ACT does xn + epilogue; per-super y drains
    (bf16, 0.5 MiB) on q10, x loads (1 MiB) on q1.  x DMA issues run
    3 supers ahead but front_stats only 2, so bn_stats never block
    the next xT copies in the DVE FIFO waiting for an unlanded x.
  * Final super runs h-major with a separate 1-bank PSUM tile per
    half (a shared tile WAR-serializes h1 matmuls behind h0's
    epilogue read), per-half drains on both rings.
  * The beta chain reads the ones-matmul total straight from PSUM
    (tensor_scalar accepts PSUM in0), skipping a serialized DVE copy
    + sem hop on the path gating the first matmul.
  * Measured: 166.6-166.8us warm clock / ~196us in the P0 power
    state (parent kernel: 171-175 / 199-202).  PE stalls 4.9us; the
    floor is ~8.5 preamble + ~24 ramp (5 MB prologue DMA + global
    beta dependency) + 124.9 PE work + ~4 tail = ~161us.
"""

import numpy as np

B, S, D, U = 4, 8192, 1024, 1024
N_CORES = 8
TOK = (B * S) // N_CORES  # 4096 tokens per core
P = 128
KB = D // P               # 8 contraction blocks
NTILES = TOK // P         # 32 token tiles per core
SUPER = 2                 # token tiles per DMA transfer (1 MiB chunks)
NJ = NTILES // SUPER      # 16 super-tiles
N_DUMMY1 = 95             # PE warmup transposes until T0's xn is ready
N_DUMMY2 = 45             # ... after the W block, before the ones-matmul
N_DUMMY3 = 20             # ... between the ones-matmul and first MM
LN_EPS = 1e-3
EPS = 1e-5

# W prologue rides both HWDGE rings (SWDGE measured ~2x slower):
#   q1 (sync):    x0, k0, k1, k2, then x1, x2, ...
#   q10 (scalar): k3, k4, k5, k6, k7, then y drains
W_SYNC = (0, 1, 2)        # q1, behind x0
W_SCALAR = (3, 4, 5, 6, 7)
DVE_ABS = (3, 4, 0)       # |W| via fused DVE stt+accum (offloads ACT)
SCRATCH_ABS = (7, 2)      # last-landing ACT chunks: abs to scratch

_NC_CACHE = {}


def _build(apply_gamma: bool, apply_beta: bool):
    """Build the single-core Bass program (SPMD: same NEFF on all 8 cores)."""
    import concourse.bacc as bacc
    import concourse.mybir as mybir
    import concourse.tile as tile
    from concourse.bass import ts
    from concourse.masks import make_identity

    fp32 = mybir.dt.float32
    bf16 = mybir.dt.bfloat16
    AF = mybir.ActivationFunctionType
    OP = mybir.AluOpType

    nc = bacc.Bacc()
    x_h = nc.dram_tensor("x", [TOK, D], fp32, kind="ExternalInput")
    w_h = nc.dram_tensor("weight", [D, U], fp32, kind="ExternalInput")
    g_h = (
        nc.dram_tensor("ln_gamma", [D], fp32, kind="ExternalInput")
        if apply_gamma
        else None
    )
    lb_h = (
        nc.dram_tensor("ln_beta", [D], fp32, kind="ExternalInput")
        if apply_beta
        else None
    )
    y_h = nc.dram_tensor("y", [TOK, U], bf16, kind="ExternalOutput")

    with tile.TileContext(nc) as tc:
        with (
            tc.tile_pool(name="singles", bufs=1) as singles,
            tc.tile_pool(name="xin", bufs=5) as xin_pool,
            tc.tile_pool(name="xn", bufs=8) as xn_pool,
            tc.tile_pool(name="xt", bufs=8) as xt_pool,
            tc.tile_pool(name="yout", bufs=3) as y_pool,
            tc.tile_pool(name="stats", bufs=8) as stats_pool,
            tc.tile_pool(name="ps_t", bufs=3, space="PSUM") as ps_t_pool,
            tc.tile_pool(name="ps_y", bufs=2, space="PSUM") as ps_y_pool,
        ):
            # ---- constants ----
            ident = singles.tile([P, P], bf16)
            make_identity(nc, ident)
            eps_t = singles.tile([P, 1], fp32)
            nc.vector.memset(eps_t, LN_EPS)
            ones_f32 = singles.tile([P, P], fp32)
            nc.vector.memset(ones_f32, 1.0)

            # ---- DMA issue order defines ring FIFO order ----
            w_view = w_h[:, :].rearrange("(ko ki) u -> ki ko u", ki=P)
            x_view = x_h[:, :].rearrange("(o p) d -> p o d", p=P)
            y_view = y_h[:, :].rearrange("(o p) u -> p o u", p=P)

            def issue_x(j, eng=None):
                x_sb = xin_pool.tile([P, SUPER, D], fp32, name="x_sb")
                (eng or nc.sync).dma_start(
                    out=x_sb, in_=x_view[:, j * SUPER : (j + 1) * SUPER, :]
                )
                return x_sb

            # q1 ring order: x0 first (its stats/xn/transposes complete
            # during the W wait in every weather), then k0, k1, k2.
            w_sb = singles.tile([P, KB, U], fp32)
            x_supers = {0: issue_x(0)}
            for k in W_SCALAR:
                nc.scalar.dma_start(out=w_sb[:, k, :], in_=w_view[:, k, :])
            for k in W_SYNC:
                nc.sync.dma_start(out=w_sb[:, k, :], in_=w_view[:, k, :])
            # (x1's issue is emitted after the ternarize trickle: the ring
            # FIFO order is identical, but the scheduler's cost model then
            # sees x1 landing late and won't hoist super-1 stats ahead of
            # the beta chain in the static DVE program.)

            if apply_gamma:
                g_sb = singles.tile([P, KB], fp32)
                nc.scalar.dma_start(
                    out=g_sb, in_=g_h[:].rearrange("(ko ki) -> ki ko", ki=P)
                )
            if apply_beta:
                lb_f32 = singles.tile([P, KB], fp32)
                nc.scalar.dma_start(
                    out=lb_f32, in_=lb_h[:].rearrange("(ko ki) -> ki ko", ki=P)
                )
                lb_sb = singles.tile([P, KB], bf16)
                nc.vector.tensor_copy(out=lb_sb, in_=lb_f32)

            # ---- W prep: sgnh on DVE, |W|+row-sum accum on ACT ----
            sgnh = singles.tile([P, KB, U], bf16)
            asum = singles.tile([P, KB], fp32)
            scratch = singles.tile([P, 2, U], fp32)
            abs_src = {}  # k -> AP holding |W| for the ternarize compare

            def emit_sgnh(k):
                # (W>=0)-0.5 in {-.5,+.5}, exact in bf16  (DVE)
                nc.vector.tensor_scalar(
                    out=sgnh[:, k, :], in0=w_sb[:, k, :], scalar1=0.0,
                    scalar2=0.5, op0=OP.is_ge, op1=OP.subtract,
                )
                if apply_gamma and not apply_beta:
                    nc.vector.tensor_scalar(
                        out=sgnh[:, k, :], in0=sgnh[:, k, :],
                        scalar1=g_sb[:, k : k + 1], scalar2=None, op0=OP.mult,
                    )

            def emit_abs(k):
                # |W| with row-sum accum.  DVE chunks: in-place abs_max
                # (0.68us vs ACT's 1.4); ACT chunks: Abs, the last-landing
                # ones to scratch so ACT needn't wait on DVE's sgnh read.
                if k in DVE_ABS and not (apply_gamma and not apply_beta):
                    # |W| = (2W) * sgnh exactly, row-sum accumulated: one
                    # fused DVE pass, in place (sgnh emitted just before;
                    # skipped when sgnh carries the ln_gamma fold)
                    nc.vector.scalar_tensor_tensor(
                        out=w_sb[:, k, :], in0=w_sb[:, k, :], scalar=2.0,
                        in1=sgnh[:, k, :], op0=OP.mult, op1=OP.mult,
                        accum_out=asum[:, k : k + 1],
                    )
                    abs_src[k] = w_sb[:, k, :]
                    return
                if k in SCRATCH_ABS:
                    i = SCRATCH_ABS.index(k)
                    dst = scratch[:, i, :]
                else:
                    dst = w_sb[:, k, :]
                nc.scalar.activation(
                    out=dst, in_=w_sb[:, k, :], func=AF.Abs,
                    accum_out=asum[:, k : k + 1],
                )
                abs_src[k] = dst

            # ---- LN stats on DVE; the normalize pass rides ACT ----
            def front_stats(x_sb, i):
                xt_ = x_sb[:, i, :]
                st = stats_pool.tile([P, 2, 6], fp32, tag="st")
                xr = xt_.rearrange("p (n f) -> p n f", f=512)
                nc.vector.bn_stats(out=st[:, 0, :], in_=xr[:, 0, :])
                nc.vector.bn_stats(out=st[:, 1, :], in_=xr[:, 1, :])
                mv = stats_pool.tile([P, 2], fp32, tag="mv")
                nc.vector.bn_aggr(out=mv, in_=st)
                nb = stats_pool.tile([P, 1], fp32, tag="nb")
                nc.vector.tensor_scalar(
                    out=nb, in0=mv[:, 0:1], scalar1=-1.0, scalar2=None,
                    op0=OP.mult,
                )
                # xn = x - mu (bf16); rsqrt scale folds into the epilogue
                xn = xn_pool.tile([P, D], bf16)
                nc.scalar.activation(
                    out=xn, in_=xt_, func=AF.Identity, bias=nb, scale=1.0
                )
                # sq = sqrt(var + eps) (tiny, ACT)
                sq = stats_pool.tile([P, 1], fp32, tag="sq")
                nc.scalar.activation(
                    out=sq, in_=mv[:, 1:2], func=AF.Sqrt, bias=eps_t, scale=1.0
                )
                return xn, sq

            # Pre-warm the SQRT ACT table (1.3us load) before it's needed
            # on the stats critical path.
            sq_warm = singles.tile([P, 1], fp32)
            nc.scalar.activation(
                out=sq_warm, in_=eps_t, func=AF.Sqrt, bias=eps_t, scale=1.0
            )

            # ---- PE warmup dummies until the first W chunk lands ----
            ps_dummy = ps_t_pool.tile([P, KB, P], bf16, tag="ps_t", name="ps_d")
            for i in range(N_DUMMY1):
                nc.tensor.transpose(ps_dummy[:, i % KB, :], ident, ident)

            def warm(k):
                # PE transposes of the just-produced sgnh chunk: warmup
                # that stretches with the actual DMA rate, so HAM stays
                # at full clock however slow the prologue runs.
                for j in range(KB):
                    nc.tensor.transpose(
                        ps_dummy[:, j, :], sgnh[:, k, ts(j, P)], ident
                    )

            # ---- transposes + copies ----
            def transpose_tile(fr):
                xn, sq = fr
                ps_xt = ps_t_pool.tile([P, KB, P], bf16, tag="ps_t")
                for k in range(KB):
                    nc.tensor.transpose(ps_xt[:, k, :], xn[:, ts(k, P)], ident)
                xT = xt_pool.tile([P, KB, P], bf16)
                nc.vector.tensor_copy(out=xT, in_=ps_xt)
                return (xT, sq)

            # Super-0 stats/transposes/copies first in the static order
            # (x0 is q1's first transfer, so they are ready before any W
            # chunk in every DMA weather), then W prep in ring-interleaved
            # landing order.  (Tested alternative: early W chunks before
            # stats -- measured 5us WORSE; the stats-first order wins.)
            frs0 = [front_stats(x_supers[0], i) for i in range(SUPER)]
            f0 = [transpose_tile(fr) for fr in frs0]
            for k in (3, 4, 0, 5, 1, 6, 2, 7):
                emit_sgnh(k)
                emit_abs(k)
                warm(k)

            asum1 = singles.tile([P, 1], fp32)
            nc.vector.tensor_reduce(
                out=asum1, in_=asum, axis=mybir.AxisListType.X, op=OP.add
            )

            for i in range(N_DUMMY2):
                nc.tensor.transpose(ps_dummy[:, i % KB, :], ident, ident)

            # cross-partition total broadcast to all partitions in ONE matmul
            ps_tot = ps_y_pool.tile([P, U], fp32, tag="ps_y", name="ps_tot")
            nc.tensor.matmul(
                ps_tot[:, 0:1], lhsT=ones_f32, rhs=asum1, start=True, stop=True
            )
            # c = (beta+EPS)/2 ;  output scale 2*beta (wq holds 0.5*w_q)
            # -- both read ps_tot straight from PSUM: one fewer serialized
            # DVE op + semaphore hop on the chain gating the first matmul
            c128 = singles.tile([P, 1], fp32)
            nc.vector.tensor_scalar(
                out=c128, in0=ps_tot[:, 0:1], scalar1=0.5 / (D * U),
                scalar2=0.5 * EPS, op0=OP.mult, op1=OP.add,
            )
            bh128 = singles.tile([P, 1], fp32)
            nc.vector.tensor_scalar(
                out=bh128, in0=ps_tot[:, 0:1], scalar1=2.0 / (D * U),
                scalar2=None, op0=OP.mult,
            )

            for i in range(N_DUMMY3):
                nc.tensor.transpose(ps_dummy[:, i % KB, :], ident, ident)

            # ---- ternarize: fused (|W| is_gt c) * sgnh per k-block ----
            wq = singles.tile([P, KB, U], bf16)  # holds 0.5*w_q (*gamma)

            def emit_tern_h(k, h):
                sl = ts(h, 512)
                nc.vector.scalar_tensor_tensor(
                    out=wq[:, k, sl], in0=abs_src[k][:, sl], scalar=c128,
                    in1=sgnh[:, k, sl], op0=OP.is_gt, op1=OP.mult,
                )

            def emit_tern(k):
                nc.vector.scalar_tensor_tensor(
                    out=wq[:, k, :], in0=abs_src[k], scalar=c128,
                    in1=sgnh[:, k, :], op0=OP.is_gt, op1=OP.mult,
                )

            # ---- back side ----
            def esc_for(sq):
                # esc = rsqrt(var+eps) * 2*beta, per token (tiny DVE chain)
                esc = stats_pool.tile([P, 1], fp32, tag="esc")
                nc.vector.reciprocal(esc, sq)
                nc.vector.tensor_scalar(
                    out=esc, in0=esc, scalar1=bh128, scalar2=None, op0=OP.mult
                )
                return esc

            beff128 = None

            def epilogue(y_sb, i, ps_y, esc, h=None):
                sl = slice(None) if h is None else ts(h, 512)
                nc.scalar.mul(out=y_sb[:, i, sl], in_=ps_y[:, sl], mul=esc)
                if apply_beta:
                    nc.vector.tensor_tensor(
                        y_sb[:, i, sl], y_sb[:, i, sl], beff128[:, sl], OP.add
                    )

            def back_tile(xt_sq, y_sb, i, j):
                xT, sq = xt_sq
                last = j == NJ - 1
                esc = esc_for(sq)
                if last:
                    # h-major: each half accumulates into its OWN 1-bank
                    # PSUM tile (a shared tile would WAR-serialize h1's
                    # matmuls behind h0's epilogue read), so each half's
                    # epilogue + drain starts as soon as it closes.  The
                    # half-tiles come from the 3-deep ps_t pool (same
                    # 1-bank size as the transpose tiles, all consumed by
                    # now) so no WAR on a recent epilogue stalls the MMs.
                    for h in range(2):
                        ps_h = ps_t_pool.tile([P, 512], fp32, tag="ps_t")
                        for k in range(KB):
                            nc.tensor.matmul(
                                ps_h,
                                lhsT=xT[:, k, :],
                                rhs=wq[:, k, ts(h, 512)],
                                start=(k == 0),
                                stop=(k == KB - 1),
                            )
                        if i == SUPER - 1 and h == 1:
                            # very last half: per-quarter epilogue+drain so
                            # the final transfer is only 128 KB and its
                            # predecessor drains during the last epilogue
                            for q in range(2):
                                sl = slice(512 + q * 256, 768 + q * 256)
                                if q == 0:
                                    nc.scalar.mul(
                                        out=y_sb[:, i, sl],
                                        in_=ps_h[:, ts(q, 256)], mul=esc,
                                    )
                                else:
                                    # second quarter's epilogue on DVE so
                                    # both quarters close in parallel
                                    nc.vector.tensor_scalar(
                                        out=y_sb[:, i, sl],
                                        in0=ps_h[:, ts(q, 256)], scalar1=esc,
                                        scalar2=None, op0=OP.mult,
                                    )
                                if apply_beta:
                                    nc.vector.tensor_tensor(
                                        y_sb[:, i, sl], y_sb[:, i, sl],
                                        beff128[:, sl], OP.add,
                                    )
                                eng = nc.scalar if q == 0 else nc.sync
                                eng.dma_start(
                                    out=y_view[:, j * SUPER + i, sl],
                                    in_=y_sb[:, i, sl],
                                )
                            continue
                        sl = ts(h, 512)
                        nc.scalar.mul(out=y_sb[:, i, sl], in_=ps_h, mul=esc)
                        if apply_beta:
                            nc.vector.tensor_tensor(
                                y_sb[:, i, sl], y_sb[:, i, sl],
                                beff128[:, sl], OP.add,
                            )
                        eng = nc.scalar if h == 0 else nc.sync
                        eng.dma_start(
                            out=y_view[:, j * SUPER + i, sl],
                            in_=y_sb[:, i, sl],
                        )
                    return
                ps_y = ps_y_pool.tile([P, U], fp32, tag="ps_y")
                for k in range(KB):
                    for h in range(2):
                        nc.tensor.matmul(
                            ps_y[:, ts(h, 512)],
                            lhsT=xT[:, k, :],
                            rhs=wq[:, k, ts(h, 512)],
                            start=(k == 0),
                            stop=(k == KB - 1),
                        )
                epilogue(y_sb, i, ps_y, esc)

            def drain_y(j, y_sb):
                if j != NJ - 1:
                    nc.scalar.dma_start(
                        out=y_view[:, j * SUPER : (j + 1) * SUPER, :], in_=y_sb
                    )

            if apply_beta:
                # beff = (ln_beta @ w_q) * 2beta, broadcast to 128 partitions
                for k in range(KB):
                    emit_tern(k)
                ps_beff = ps_y_pool.tile([P, U], fp32, tag="ps_y", name="ps_bf")
                for k in range(KB):
                    for h in range(2):
                        nc.tensor.matmul(
                            ps_beff[0:1, ts(h, 512)],
                            lhsT=lb_sb[:, k : k + 1],
                            rhs=wq[:, k, ts(h, 512)],
                            start=(k == 0),
                            stop=(k == KB - 1),
                        )
                beff = singles.tile([1, U], fp32)
                nc.vector.tensor_scalar(
                    out=beff, in0=ps_beff[0:1, :], scalar1=bh128[0:1, 0:1],
                    scalar2=None, op0=OP.mult,
                )
                ps_b2 = ps_y_pool.tile([P, U], fp32, tag="ps_y")
                ones_row = singles.tile([1, P], fp32)
                nc.vector.memset(ones_row, 1.0)
                for h in range(2):
                    nc.tensor.matmul(
                        ps_b2[:, ts(h, 512)], lhsT=ones_row,
                        rhs=beff[:, ts(h, 512)], start=True, stop=True,
                    )
                beff128 = singles.tile([P, U], bf16)
                nc.vector.tensor_copy(out=beff128, in_=ps_b2)
                if apply_gamma:
                    for k in range(KB):
                        nc.vector.tensor_scalar(
                            out=wq[:, k, :], in0=wq[:, k, :],
                            scalar1=g_sb[:, k : k + 1], scalar2=None,
                            op0=OP.mult,
                        )

            # ---- first super: ternarize trickles per k, matmuls follow
            # k-by-k; super-1 stats/transposes slot into the gaps.
            x_supers[1] = issue_x(1)
            y_sb0 = y_pool.tile([P, SUPER, U], bf16)
            (xtA, sqA), (xtB, sqB) = f0
            psA = ps_y_pool.tile([P, U], fp32, tag="ps_y")
            psB = ps_y_pool.tile([P, U], fp32, tag="ps_y")
            fr1 = [None, None]
            f1 = [None, None]
            for k in range(KB):
                # ternarize per u-half so the first consumable wq slice
                # lands in ~0.7us instead of ~1.3
                for h in range(2):
                    if not apply_beta:
                        emit_tern_h(k, h)
                    for ps, xt in ((psA, xtA), (psB, xtB)):
                        nc.tensor.matmul(
                            ps[:, ts(h, 512)],
                            lhsT=xt[:, k, :],
                            rhs=wq[:, k, ts(h, 512)],
                            start=(k == 0),
                            stop=(k == KB - 1),
                        )
                # one super-1 stats block rides the trickle; the rest of
                # super-1's front work follows the last ternarize so the
                # DVE-serial tern stream stays dense.
                if k == 5:
                    fr1[0] = front_stats(x_supers[1], 0)
            escA, escB = esc_for(sqA), esc_for(sqB)
            epilogue(y_sb0, 0, psA, escA)
            f1[0] = transpose_tile(fr1[0])
            fr1[1] = front_stats(x_supers[1], 1)
            epilogue(y_sb0, 1, psB, escB)
            f1[1] = transpose_tile(fr1[1])
            drain_y(0, y_sb0)
            fronts = {1: f1}

            x_supers[2] = issue_x(2)
            x_supers[3] = issue_x(3)
            frs = {2: [front_stats(x_supers[2], i) for i in range(SUPER)]}

            # ---- steady loop, per-tile interleave: M(j)i then T(j+1)i.
            # x DMA is issued 3 supers ahead but front_stats only 2 ahead,
            # so the bn_stats never wait on an unlanded x super while the
            # next xT copies sit behind them in the DVE FIFO.
            for j in range(1, NJ):
                y_sb = y_pool.tile([P, SUPER, U], bf16)
                xts = fronts.pop(j)
                nxt = [] if j + 1 < NJ else None
                for i in range(SUPER):
                    back_tile(xts[i], y_sb, i, j)
                    if nxt is not None:
                        nxt.append(transpose_tile(frs[j + 1][i]))
                if nxt is not None:
                    del frs[j + 1]
                    fronts[j + 1] = nxt
                drain_y(j, y_sb)
                if j + 3 < NJ:
                    x_supers[j + 3] = issue_x(j + 3)
                if j + 2 < NJ:
                    frs[j + 2] = [
                        front_stats(x_supers[j + 2], i) for i in range(SUPER)
                    ]

    nc.compile()
    return nc


def _get_nc(apply_gamma: bool, apply_beta: bool):
    key = (apply_gamma, apply_beta)
    if key not in _NC_CACHE:
        _NC_CACHE[key] = _build(apply_gamma, apply_beta)
    return _NC_CACHE[key]


def _make_in_maps(x, w, g, lb, apply_gamma, apply_beta):
    xf = np.ascontiguousarray(x.reshape(B * S, D))
    in_maps = []
    for c in range(N_CORES):
        m = {
            "x": np.ascontiguousarray(xf[c * TOK : (c + 1) * TOK]),
            "weight": w,
        }
        if apply_gamma:
            m["ln_gamma"] = g
        if apply_beta:
            m["ln_beta"] = lb
        in_maps.append(m)
    return in_maps


def run(inputs, trace=False, tmpdir=None):
    """Shard, run on 8 cores, gather. Returns (y, BassKernelResults)."""
    from concourse.bass_utils import run_bass_kernel_spmd

    x = np.asarray(inputs["x"], dtype=np.float32)
    w = np.ascontiguousarray(np.asarray(inputs["weight"], dtype=np.float32))
    g = np.ascontiguousarray(np.asarray(inputs["ln_gamma"], dtype=np.float32))
    lb = np.ascontiguousarray(np.asarray(inputs["ln_beta"], dtype=np.float32))
    apply_gamma = not bool(np.all(g == 1.0))
    apply_beta = not bool(np.all(lb == 0.0))

    nc = _get_nc(apply_gamma, apply_beta)
    in_maps = _make_in_maps(x, w, g, lb, apply_gamma, apply_beta)
    res = run_bass_kernel_spmd(
        nc, in_maps, core_ids=list(range(N_CORES)), trace=trace, tmpdir=tmpdir
    )
    y = np.concatenate(
        [np.asarray(r["y"]).astype(np.float32) for r in res.results], axis=0
    )
    return y.reshape(B, S, U), res


def kernel(**inputs) -> np.ndarray:
    y, _ = run(inputs, trace=False)
    return y


# revision 58
# speedup vs baseline: 1.0257x; 1.0165x over previous
"""Trainium2 Bass kernel for nn_BitLinear (LayerNorm -> 1.58-bit BitLinear).

Math notes
----------
Reference computes, per the module:
    xn    = LN(x) * ln_gamma + ln_beta            (eps = 1e-3)
    beta  = mean(|W|);  w_q = clip(round(W / (beta + 1e-5)), -1, 1)
    gamma = max(|xn|)   (global absmax)
    xq    = clip(xn * 128 / gamma, -128 + 1e-5, 128 - 1e-5)
    y     = (xq @ w_q) * (gamma * beta / 128)

The gamma factor cancels exactly: (xn*128/gamma) @ w_q * (gamma*beta/128)
== (xn @ w_q) * beta.  The clip only affects elements within relative
7.8e-8 of the global absmax -- far below f32 matmul roundoff.  So the
kernel computes y = (LN(x) @ w_q) * beta, fully data-parallel over
tokens (no collectives).

w_q is ternary: w_q = sign(W) * 1[|W| > c] with c = 0.5*(beta+1e-5).
The kernel stores wq' = 0.5*w_q via one fused DVE op per k-block:
    wq = (|W| is_gt c) * sgnh,   sgnh = (W>=0)-0.5 in {-.5,+.5}
(the 2x is folded into the output scale 2*beta).  All compares are f32:
a bf16 compare would misclassify ~300 weights near the threshold.

LN normalization scale folds into the epilogue: xn = (x - mu) in bf16,
and esc[t] = rsqrt(var+eps)[t] * 2*beta scales each output row.

Sharding: data-parallel over the 32768 tokens, 4096 per core; weight
replicated.  y is written bf16 (halves drain traffic; ~1e-3 extra
rel-err) and upcast to f32 on the host.

Schedule (v2, rebuilt from trace analysis of the previous kernel):
  * Measured engine rates: MM N=512 gap 216ns warm / 259ns in the P0
    power state (run-to-run chip power state; uncontrollable); PE
    transposes 56ns; DVE f32 pass over [128,1024] ~0.7us; ACT pass
    ~1.1us; HWDGE rings 125-245 B/ns each (HBM-stack contention with
    the other 7 cores' prologues); NEFF preamble ~8.5us.  Per-engine
    instruction order is STATIC (fixed by Tile's cost-model
    simulation), so emission order must be correct for slow AND fast
    DMA weather -- the runtime does not re-dispatch by readiness.
  * Prologue DMA: q1 (sync): x0, k0, k1, k2, x1, x2...
                  q10 (scalar): k3..k7, then y drains.
    (gpsimd SWDGE measured ~2x slower -- not used.)
  * W prep: super-0 stats/xn/transposes/copies first in the static
    order (x0 is q1's first transfer -- always ready first), then
    per-chunk sgnh (DVE) + |W|+row-sum (3 early chunks fused on DVE
    in-place, 5 on ACT, the last-landing two to scratch), in
    ring-interleaved landing order so a slow ring costs at most one
    cross-ring head-of-line wait.  After each chunk's sgnh the PE
    transposes it into ps_dummy: warmup that stretches with the
    actual DMA rate, keeping the HAM clock warm however slow the
    prologue runs.  beta -> c closes ~2us after the last W byte.
  * Ternarize trickles per k (fused DVE stt ~1.3us/k) interleaved
    with the first super's matmuls k-by-k (first MM ~33us vs 42 in
    the parent kernel; x1's DMA issue is emitted after the trickle so
    the scheduler cannot hoist super-1 stats ahead of the beta chain).
  * Steady loop per tile: M(j)i then T(j+1)i; xT PSUM->SBUF copies on
    DVE during back(j); ACT does xn + epilogue; per-super y drains
    (bf16, 0.5 MiB) on q10, x loads (1 MiB) on q1.  x DMA issues run
    3 supers ahead but front_stats only 2, so bn_stats never block
    the next xT copies in the DVE FIFO waiting for an unlanded x.
  * Final super runs h-major with a separate 1-bank PSUM tile per
    half (a shared tile WAR-serializes h1 matmuls behind h0's
    epilogue read), per-half drains on both rings.
  * The beta chain reads the ones-matmul total straight from PSUM
    (tensor_scalar accepts PSUM in0), skipping a serialized DVE copy
    + sem hop on the path gating the first matmul.
  * Measured: 166.6-166.8us warm clock / ~196us in the P0 power
    state (parent kernel: 171-175 / 199-202).  PE stalls 4.9us; the
    floor is ~8.5 preamble + ~24 ramp (5 MB prologue DMA + global
    beta dependency) + 124.9 PE work + ~4 tail = ~161us.
"""

import numpy as np

B, S, D, U = 4, 8192, 1024, 1024
N_CORES = 8
TOK = (B * S) // N_CORES  # 4096 tokens per core
P = 128
KB = D // P               # 8 contraction blocks
NTILES = TOK // P         # 32 token tiles per core
SUPER = 2                 # token tiles per DMA transfer (1 MiB chunks)
NJ = NTILES // SUPER      # 16 super-tiles
N_DUMMY1 = 95             # PE warmup transposes until T0's xn is ready
N_DUMMY2 = 45             # ... after the W block, before the ones-matmul
N_DUMMY3 = 20             # ... between the ones-matmul and first MM
LN_EPS = 1e-3
EPS = 1e-5

# W prologue rides both HWDGE rings (SWDGE measured ~2x slower):
#   q1 (sync):    x0, k0, k1, k2, then x1, x2, ...
#   q10 (scalar): k3, k4, k5, k6, k7, then y drains
W_SYNC = (0, 1, 2)        # q1, behind x0
W_SCALAR = (3, 4, 5, 6, 7)
DVE_ABS = (3, 4, 7)       # fused DVE stt+accum; k7 (last-landing) on
                          # DVE so the final abs tail runs on BOTH engines
SCRATCH_ABS = (6, 2)      # last-landing ACT chunks: abs to scratch

_NC_CACHE = {}


def _build(apply_gamma: bool, apply_beta: bool):
    """Build the single-core Bass program (SPMD: same NEFF on all 8 cores)."""
    import concourse.bacc as bacc
    import concourse.mybir as mybir
    import concourse.tile as tile
    from concourse.bass import ts
    from concourse.masks import make_identity

    fp32 = mybir.dt.float32
    bf16 = mybir.dt.bfloat16
    AF = mybir.ActivationFunctionType
    OP = mybir.AluOpType

    nc = bacc.Bacc()
    x_h = nc.dram_tensor("x", [TOK, D], fp32, kind="ExternalInput")
    w_h = nc.dram_tensor("weight", [D, U], fp32, kind="ExternalInput")
    g_h = (
        nc.dram_tensor("ln_gamma", [D], fp32, kind="ExternalInput")
        if apply_gamma
        else None
    )
    lb_h = (
        nc.dram_tensor("ln_beta", [D], fp32, kind="ExternalInput")
        if apply_beta
        else None
    )
    y_h = nc.dram_tensor("y", [TOK, U], bf16, kind="ExternalOutput")

    with tile.TileContext(nc) as tc:
        with (
            tc.tile_pool(name="singles", bufs=1) as singles,
            tc.tile_pool(name="xin", bufs=5) as xin_pool,
            tc.tile_pool(name="xn", bufs=8) as xn_pool,
            tc.tile_pool(name="xt", bufs=8) as xt_pool,
            tc.tile_pool(name="yout", bufs=3) as y_pool,
            tc.tile_pool(name="stats", bufs=8) as stats_pool,
            tc.tile_pool(name="ps_t", bufs=3, space="PSUM") as ps_t_pool,
            tc.tile_pool(name="ps_y", bufs=2, space="PSUM") as ps_y_pool,
        ):
            # ---- constants ----
            ident = singles.tile([P, P], bf16)
            make_identity(nc, ident)
            eps_t = singles.tile([P, 1], fp32)
            nc.vector.memset(eps_t, LN_EPS)
            ones_f32 = singles.tile([P, P], fp32)
            nc.vector.memset(ones_f32, 1.0)

            # ---- DMA issue order defines ring FIFO order ----
            w_view = w_h[:, :].rearrange("(ko ki) u -> ki ko u", ki=P)
            x_view = x_h[:, :].rearrange("(o p) d -> p o d", p=P)
            y_view = y_h[:, :].rearrange("(o p) u -> p o u", p=P)

            def issue_x(j, eng=None):
                x_sb = xin_pool.tile([P, SUPER, D], fp32, name="x_sb")
                (eng or nc.sync).dma_start(
                    out=x_sb, in_=x_view[:, j * SUPER : (j + 1) * SUPER, :]
                )
                return x_sb

            # q1 ring order: x0 first (its stats/xn/transposes complete
            # during the W wait in every weather), then k0, k1, k2.
            w_sb = singles.tile([P, KB, U], fp32)
            x_supers = {0: issue_x(0)}
            for k in W_SCALAR:
                nc.scalar.dma_start(out=w_sb[:, k, :], in_=w_view[:, k, :])
            for k in W_SYNC:
                nc.sync.dma_start(out=w_sb[:, k, :], in_=w_view[:, k, :])
            # (x1's issue is emitted after the ternarize trickle: the ring
            # FIFO order is identical, but the scheduler's cost model then
            # sees x1 landing late and won't hoist super-1 stats ahead of
            # the beta chain in the static DVE program.)

            if apply_gamma:
                g_sb = singles.tile([P, KB], fp32)
                nc.scalar.dma_start(
                    out=g_sb, in_=g_h[:].rearrange("(ko ki) -> ki ko", ki=P)
                )
            if apply_beta:
                lb_f32 = singles.tile([P, KB], fp32)
                nc.scalar.dma_start(
                    out=lb_f32, in_=lb_h[:].rearrange("(ko ki) -> ki ko", ki=P)
                )
                lb_sb = singles.tile([P, KB], bf16)
                nc.vector.tensor_copy(out=lb_sb, in_=lb_f32)

            # ---- W prep: sgnh on DVE, |W|+row-sum accum on ACT ----
            sgnh = singles.tile([P, KB, U], bf16)
            asum = singles.tile([P, KB], fp32)
            scratch = singles.tile([P, 2, U], fp32)
            abs_src = {}  # k -> AP holding |W| for the ternarize compare

            def emit_sgnh(k):
                # (W>=0)-0.5 in {-.5,+.5}, exact in bf16  (DVE)
                nc.vector.tensor_scalar(
                    out=sgnh[:, k, :], in0=w_sb[:, k, :], scalar1=0.0,
                    scalar2=0.5, op0=OP.is_ge, op1=OP.subtract,
                )
                if apply_gamma and not apply_beta:
                    nc.vector.tensor_scalar(
                        out=sgnh[:, k, :], in0=sgnh[:, k, :],
                        scalar1=g_sb[:, k : k + 1], scalar2=None, op0=OP.mult,
                    )

            def emit_abs(k):
                # |W| with row-sum accum.  DVE chunks: in-place abs_max
                # (0.68us vs ACT's 1.4); ACT chunks: Abs, the last-landing
                # ones to scratch so ACT needn't wait on DVE's sgnh read.
                if k in DVE_ABS and not (apply_gamma and not apply_beta):
                    # |W| = (2W) * sgnh exactly, row-sum accumulated: one
                    # fused DVE pass, in place (sgnh emitted just before;
                    # skipped when sgnh carries the ln_gamma fold)
                    nc.vector.scalar_tensor_tensor(
                        out=w_sb[:, k, :], in0=w_sb[:, k, :], scalar=2.0,
                        in1=sgnh[:, k, :], op0=OP.mult, op1=OP.mult,
                        accum_out=asum[:, k : k + 1],
                    )
                    abs_src[k] = w_sb[:, k, :]
                    return
                if k in SCRATCH_ABS:
                    i = SCRATCH_ABS.index(k)
                    dst = scratch[:, i, :]
                else:
                    dst = w_sb[:, k, :]
                nc.scalar.activation(
                    out=dst, in_=w_sb[:, k, :], func=AF.Abs,
                    accum_out=asum[:, k : k + 1],
                )
                abs_src[k] = dst

            # ---- LN stats on DVE; the normalize pass rides ACT ----
            def front_stats(x_sb, i):
                xt_ = x_sb[:, i, :]
                st = stats_pool.tile([P, 2, 6], fp32, tag="st")
                xr = xt_.rearrange("p (n f) -> p n f", f=512)
                nc.vector.bn_stats(out=st[:, 0, :], in_=xr[:, 0, :])
                nc.vector.bn_stats(out=st[:, 1, :], in_=xr[:, 1, :])
                mv = stats_pool.tile([P, 2], fp32, tag="mv")
                nc.vector.bn_aggr(out=mv, in_=st)
                nb = stats_pool.tile([P, 1], fp32, tag="nb")
                nc.vector.tensor_scalar(
                    out=nb, in0=mv[:, 0:1], scalar1=-1.0, scalar2=None,
                    op0=OP.mult,
                )
                # xn = x - mu (bf16); rsqrt scale folds into the epilogue
                xn = xn_pool.tile([P, D], bf16)
                nc.scalar.activation(
                    out=xn, in_=xt_, func=AF.Identity, bias=nb, scale=1.0
                )
                # sq = sqrt(var + eps) (tiny, ACT)
                sq = stats_pool.tile([P, 1], fp32, tag="sq")
                nc.scalar.activation(
                    out=sq, in_=mv[:, 1:2], func=AF.Sqrt, bias=eps_t, scale=1.0
                )
                return xn, sq

            # Pre-warm the SQRT ACT table (1.3us load) before it's needed
            # on the stats critical path.
            sq_warm = singles.tile([P, 1], fp32)
            nc.scalar.activation(
                out=sq_warm, in_=eps_t, func=AF.Sqrt, bias=eps_t, scale=1.0
            )

            # ---- PE warmup dummies until the first W chunk lands ----
            ps_dummy = ps_t_pool.tile([P, KB, P], bf16, tag="ps_t", name="ps_d")
            for i in range(N_DUMMY1):
                nc.tensor.transpose(ps_dummy[:, i % KB, :], ident, ident)

            def warm(k):
                # PE transposes of the just-produced sgnh chunk: warmup
                # that stretches with the actual DMA rate, so HAM stays
                # at full clock however slow the prologue runs.
                for j in range(KB):
                    nc.tensor.transpose(
                        ps_dummy[:, j, :], sgnh[:, k, ts(j, P)], ident
                    )

            # ---- transposes + copies ----
            def transpose_tile(fr):
                xn, sq = fr
                ps_xt = ps_t_pool.tile([P, KB, P], bf16, tag="ps_t")
                for k in range(KB):
                    nc.tensor.transpose(ps_xt[:, k, :], xn[:, ts(k, P)], ident)
                xT = xt_pool.tile([P, KB, P], bf16)
                nc.vector.tensor_copy(out=xT, in_=ps_xt)
                return (xT, sq)

            # Super-0 stats/transposes/copies first in the static order
            # (x0 is q1's first transfer, so they are ready before any W
            # chunk in every DMA weather), then W prep in ring-interleaved
            # landing order.  (Tested alternative: early W chunks before
            # stats -- measured 5us WORSE; the stats-first order wins.)
            frs0 = [front_stats(x_supers[0], i) for i in range(SUPER)]
            f0 = [transpose_tile(fr) for fr in frs0]
            for k in (3, 4, 0, 5, 1, 6, 2, 7):
                emit_sgnh(k)
                emit_abs(k)
                warm(k)

            asum1 = singles.tile([P, 1], fp32)
            nc.vector.tensor_reduce(
                out=asum1, in_=asum, axis=mybir.AxisListType.X, op=OP.add
            )

            for i in range(N_DUMMY2):
                nc.tensor.transpose(ps_dummy[:, i % KB, :], ident, ident)

            # cross-partition total broadcast to all partitions in ONE matmul
            ps_tot = ps_y_pool.tile([P, U], fp32, tag="ps_y", name="ps_tot")
            nc.tensor.matmul(
                ps_tot[:, 0:1], lhsT=ones_f32, rhs=asum1, start=True, stop=True
            )
            # c = (beta+EPS)/2 ;  output scale 2*beta (wq holds 0.5*w_q)
            # -- both read ps_tot straight from PSUM: one fewer serialized
            # DVE op + semaphore hop on the chain gating the first matmul
            c128 = singles.tile([P, 1], fp32)
            nc.vector.tensor_scalar(
                out=c128, in0=ps_tot[:, 0:1], scalar1=0.5 / (D * U),
                scalar2=0.5 * EPS, op0=OP.mult, op1=OP.add,
            )
            bh128 = singles.tile([P, 1], fp32)
            nc.vector.tensor_scalar(
                out=bh128, in0=ps_tot[:, 0:1], scalar1=2.0 / (D * U),
                scalar2=None, op0=OP.mult,
            )

            for i in range(N_DUMMY3):
                nc.tensor.transpose(ps_dummy[:, i % KB, :], ident, ident)

            # ---- ternarize: fused (|W| is_gt c) * sgnh per k-block ----
            wq = singles.tile([P, KB, U], bf16)  # holds 0.5*w_q (*gamma)

            def emit_tern_h(k, h):
                sl = ts(h, 512)
                nc.vector.scalar_tensor_tensor(
                    out=wq[:, k, sl], in0=abs_src[k][:, sl], scalar=c128,
                    in1=sgnh[:, k, sl], op0=OP.is_gt, op1=OP.mult,
                )

            def emit_tern(k):
                nc.vector.scalar_tensor_tensor(
                    out=wq[:, k, :], in0=abs_src[k], scalar=c128,
                    in1=sgnh[:, k, :], op0=OP.is_gt, op1=OP.mult,
                )

            # ---- back side ----
            def esc_for(sq):
                # esc = rsqrt(var+eps) * 2*beta, per token (tiny DVE chain)
                esc = stats_pool.tile([P, 1], fp32, tag="esc")
                nc.vector.reciprocal(esc, sq)
                nc.vector.tensor_scalar(
                    out=esc, in0=esc, scalar1=bh128, scalar2=None, op0=OP.mult
                )
                return esc

            beff128 = None

            def epilogue(y_sb, i, ps_y, esc, h=None):
                sl = slice(None) if h is None else ts(h, 512)
                nc.scalar.mul(out=y_sb[:, i, sl], in_=ps_y[:, sl], mul=esc)
                if apply_beta:
                    nc.vector.tensor_tensor(
                        y_sb[:, i, sl], y_sb[:, i, sl], beff128[:, sl], OP.add
                    )

            def back_tile(xt_sq, y_sb, i, j):
                xT, sq = xt_sq
                last = j == NJ - 1
                esc = esc_for(sq)
                if last:
                    # h-major: each half accumulates into its OWN 1-bank
                    # PSUM tile (a shared tile would WAR-serialize h1's
                    # matmuls behind h0's epilogue read), so each half's
                    # epilogue + drain starts as soon as it closes.  The
                    # half-tiles come from the 3-deep ps_t pool (same
                    # 1-bank size as the transpose tiles, all consumed by
                    # now) so no WAR on a recent epilogue stalls the MMs.
                    for h in range(2):
                        ps_h = ps_t_pool.tile([P, 512], fp32, tag="ps_t")
                        for k in range(KB):
                            nc.tensor.matmul(
                                ps_h,
                                lhsT=xT[:, k, :],
                                rhs=wq[:, k, ts(h, 512)],
                                start=(k == 0),
                                stop=(k == KB - 1),
                            )
                        if i == SUPER - 1 and h == 1:
                            # very last half: per-quarter epilogue+drain so
                            # the final transfer is only 128 KB and its
                            # predecessor drains during the last epilogue
                            for q in range(2):
                                sl = slice(512 + q * 256, 768 + q * 256)
                                if q == 0:
                                    nc.scalar.mul(
                                        out=y_sb[:, i, sl],
                                        in_=ps_h[:, ts(q, 256)], mul=esc,
                                    )
                                else:
                                    # second quarter's epilogue on DVE so
                                    # both quarters close in parallel
                                    nc.vector.tensor_scalar(
                                        out=y_sb[:, i, sl],
                                        in0=ps_h[:, ts(q, 256)], scalar1=esc,
                                        scalar2=None, op0=OP.mult,
                                    )
                                if apply_beta:
                                    nc.vector.tensor_tensor(
                                        y_sb[:, i, sl], y_sb[:, i, sl],
                                        beff128[:, sl], OP.add,
                                    )
                                eng = nc.scalar if q == 0 else nc.sync
                                eng.dma_start(
                                    out=y_view[:, j * SUPER + i, sl],
                                    in_=y_sb[:, i, sl],
                                )
                            continue
                        sl = ts(h, 512)
                        nc.scalar.mul(out=y_sb[:, i, sl], in_=ps_h, mul=esc)
                        if apply_beta:
                            nc.vector.tensor_tensor(
                                y_sb[:, i, sl], y_sb[:, i, sl],
                                beff128[:, sl], OP.add,
                            )
                        eng = nc.scalar if h == 0 else nc.sync
                        eng.dma_start(
                            out=y_view[:, j * SUPER + i, sl],
                            in_=y_sb[:, i, sl],
                        )
                    return
                ps_y = ps_y_pool.tile([P, U], fp32, tag="ps_y")
                for k in range(KB):
                    for h in range(2):
                        nc.tensor.matmul(
                            ps_y[:, ts(h, 512)],
                            lhsT=xT[:, k, :],
                            rhs=wq[:, k, ts(h, 512)],
                            start=(k == 0),
                            stop=(k == KB - 1),
                        )
                epilogue(y_sb, i, ps_y, esc)

            def drain_y(j, y_sb):
                if j != NJ - 1:
                    nc.scalar.dma_start(
                        out=y_view[:, j * SUPER : (j + 1) * SUPER, :], in_=y_sb
                    )

            if apply_beta:
                # beff = (ln_beta @ w_q) * 2beta, broadcast to 128 partitions
                for k in range(KB):
                    emit_tern(k)
                ps_beff = ps_y_pool.tile([P, U], fp32, tag="ps_y", name="ps_bf")
                for k in range(KB):
                    for h in range(2):
                        nc.tensor.matmul(
                            ps_beff[0:1, ts(h, 512)],
                            lhsT=lb_sb[:, k : k + 1],
                            rhs=wq[:, k, ts(h, 512)],
                            start=(k == 0),
                            stop=(k == KB - 1),
                        )
                beff = singles.tile([1, U], fp32)
                nc.vector.tensor_scalar(
                    out=beff, in0=ps_beff[0:1, :], scalar1=bh128[0:1, 0:1],
                    scalar2=None, op0=OP.mult,
                )
                ps_b2 = ps_y_pool.tile([P, U], fp32, tag="ps_y")
                ones_row = singles.tile([1, P], fp32)
                nc.vector.memset(ones_row, 1.0)
                for h in range(2):
                    nc.tensor.matmul(
                        ps_b2[:, ts(h, 512)], lhsT=ones_row,
                        rhs=beff[:, ts(h, 512)], start=True, stop=True,
                    )
                beff128 = singles.tile([P, U], bf16)
                nc.vector.tensor_copy(out=beff128, in_=ps_b2)
                if apply_gamma:
                    for k in range(KB):
                        nc.vector.tensor_scalar(
                            out=wq[:, k, :], in0=wq[:, k, :],
                            scalar1=g_sb[:, k : k + 1], scalar2=None,
                            op0=OP.mult,
                        )

            # ---- first super: ternarize trickles per k, matmuls follow
            # k-by-k; super-1 stats/transposes slot into the gaps.
            x_supers[1] = issue_x(1)
            y_sb0 = y_pool.tile([P, SUPER, U], bf16)
            (xtA, sqA), (xtB, sqB) = f0
            psA = ps_y_pool.tile([P, U], fp32, tag="ps_y")
            psB = ps_y_pool.tile([P, U], fp32, tag="ps_y")
            fr1 = [None, None]
            f1 = [None, None]
            for k in range(KB):
                # ternarize per u-half so the first consumable wq slice
                # lands in ~0.7us instead of ~1.3
                for h in range(2):
                    if not apply_beta:
                        emit_tern_h(k, h)
                    for ps, xt in ((psA, xtA), (psB, xtB)):
                        nc.tensor.matmul(
                            ps[:, ts(h, 512)],
                            lhsT=xt[:, k, :],
                            rhs=wq[:, k, ts(h, 512)],
                            start=(k == 0),
                            stop=(k == KB - 1),
                        )
                # one super-1 stats block rides the trickle; the rest of
                # super-1's front work follows the last ternarize so the
                # DVE-serial tern stream stays dense.
                if k == 5:
                    fr1[0] = front_stats(x_supers[1], 0)
            escA, escB = esc_for(sqA), esc_for(sqB)
            epilogue(y_sb0, 0, psA, escA)
            f1[0] = transpose_tile(fr1[0])
            fr1[1] = front_stats(x_supers[1], 1)
            epilogue(y_sb0, 1, psB, escB)
            f1[1] = transpose_tile(fr1[1])
            drain_y(0, y_sb0)
            fronts = {1: f1}

            x_supers[2] = issue_x(2)
            x_supers[3] = issue_x(3)
            frs = {2: [front_stats(x_supers[2], i) for i in range(SUPER)]}

            # ---- steady loop, per-tile interleave: M(j)i then T(j+1)i.
            # x DMA is issued 3 supers ahead but front_stats only 2 ahead,
            # so the bn_stats never wait on an unlanded x super while the
            # next xT copies sit behind them in the DVE FIFO.
            for j in range(1, NJ):
                y_sb = y_pool.tile([P, SUPER, U], bf16)
                xts = fronts.pop(j)
                nxt = [] if j + 1 < NJ else None
                for i in range(SUPER):
                    back_tile(xts[i], y_sb, i, j)
                    if nxt is not None:
                        nxt.append(transpose_tile(frs[j + 1][i]))
                if nxt is not None:
                    del frs[j + 1]
                    fronts[j + 1] = nxt
                drain_y(j, y_sb)
                if j + 3 < NJ:
                    x_supers[j + 3] = issue_x(j + 3)
                if j + 2 < NJ:
                    frs[j + 2] = [
                        front_stats(x_supers[j + 2], i) for i in range(SUPER)
                    ]

    nc.compile()
    return nc


def _get_nc(apply_gamma: bool, apply_beta: bool):
    key = (apply_gamma, apply_beta)
    if key not in _NC_CACHE:
        _NC_CACHE[key] = _build(apply_gamma, apply_beta)
    return _NC_CACHE[key]


def _make_in_maps(x, w, g, lb, apply_gamma, apply_beta):
    xf = np.ascontiguousarray(x.reshape(B * S, D))
    in_maps = []
    for c in range(N_CORES):
        m = {
            "x": np.ascontiguousarray(xf[c * TOK : (c + 1) * TOK]),
            "weight": w,
        }
        if apply_gamma:
            m["ln_gamma"] = g
        if apply_beta:
            m["ln_beta"] = lb
        in_maps.append(m)
    return in_maps


def run(inputs, trace=False, tmpdir=None):
    """Shard, run on 8 cores, gather. Returns (y, BassKernelResults)."""
    from concourse.bass_utils import run_bass_kernel_spmd

    x = np.asarray(inputs["x"], dtype=np.float32)
    w = np.ascontiguousarray(np.asarray(inputs["weight"], dtype=np.float32))
    g = np.ascontiguousarray(np.asarray(inputs["ln_gamma"], dtype=np.float32))
    lb = np.ascontiguousarray(np.asarray(inputs["ln_beta"], dtype=np.float32))
    apply_gamma = not bool(np.all(g == 1.0))
    apply_beta = not bool(np.all(lb == 0.0))

    nc = _get_nc(apply_gamma, apply_beta)
    in_maps = _make_in_maps(x, w, g, lb, apply_gamma, apply_beta)
    res = run_bass_kernel_spmd(
        nc, in_maps, core_ids=list(range(N_CORES)), trace=trace, tmpdir=tmpdir
    )
    y = np.concatenate(
        [np.asarray(r["y"]).astype(np.float32) for r in res.results], axis=0
    )
    return y.reshape(B, S, U), res


def kernel(**inputs) -> np.ndarray:
    y, _ = run(inputs, trace=False)
    return y
